# revision 3
# baseline (speedup 1.0000x reference)
"""Pairwise squared-Euclidean distance map on 8 TRN2 NeuronCores.

d[b, i, j] = sum_c (a[b, c, i] - b[b, c, j])^2
           = aa[b, i] + bb[b, j] - 2 * <a[b, :, i], b[b, :, j]>

Sharding: data-parallel over the N dimension (rows of the distance map).
Core k computes d[:, k*512:(k+1)*512, :] from a[:, :, k*512:(k+1)*512]
and the full (small) b tensor.

Per-core kernel: one augmented matmul per output tile computes the full
expression in a single PE pass. With
    lhsT = [ -2*a  ;  aa  ;  1 ]   (K = C+2 = 66 partitions, 128 N cols)
    rhs  = [   b   ;  1   ;  bb ]  (66 partitions, 512 M cols)
we get lhsT.T @ rhs = -2*a.b + aa + bb = d directly in PSUM; the only
post-processing is a plain PSUM->SBUF copy (split across the Vector and
Scalar engines) before the store DMA. Matmuls run in float32r (full-rate
fp32 at moving-dim >= 256).

The aa/bb norm rows are computed on-device: square the inputs (ScalarE),
append a ones row at partition 64, then contract with a constant [65, 2]
selector as matmul lhsT so one PE pass emits the [aa; ones] (or
[ones; bb]) row pair directly — engine APs may only start at partitions
{0, 32, 64, 96}, so the pair is written with a single base-64 copy.
"""

import numpy as np
from contextlib import ExitStack

import concourse.bass as bass
import concourse.bacc as bacc
import concourse.mybir as mybir
from concourse.tile import TileContext
from concourse.bass_utils import run_bass_kernel_spmd

B, C, N, M = 4, 64, 4096, 4096
NCORES = 8
NSH = N // NCORES          # 512 N rows per core
NB = NSH // 128            # 4 row blocks of 128
MC = 512                   # matmul moving free dim (one PSUM bank of fp32)
PSUM_W = 1024              # main PSUM tile width (2 banks, 2 matmuls)
KAUG = C + 2               # contraction dim with the two norm rows

F32 = mybir.dt.float32
F32R = mybir.dt.float32r

_CACHE = {}


def _build_nc():
    nc = bacc.Bacc(
        "TRN2",
        target_bir_lowering=False,
        debug=False,
        enable_asserts=True,
        num_devices=NCORES,
    )
    a_d = nc.declare_dram_parameter("a", [B, C, NSH], F32, isOutput=False)
    b_d = nc.declare_dram_parameter("b", [B, C, M], F32R, isOutput=False)
    ones_d = nc.declare_dram_parameter("ones", [M], F32R, isOutput=False)
    # selector columns (see _make_in_maps): 0: pick-ones, 1: sum-squares,
    # 2: sum-squares, 3: pick-ones
    sel_d = nc.declare_dram_parameter("sel", [C + 1, 4], F32R, isOutput=False)
    d_d = nc.declare_dram_parameter("d", [B, NSH, M], F32, isOutput=True)

    with ExitStack() as ctx:
        tc = ctx.enter_context(TileContext(nc))
        const = ctx.enter_context(tc.tile_pool(name="const", bufs=1))
        bpool = ctx.enter_context(tc.tile_pool(name="baug", bufs=2))
        apool = ctx.enter_context(tc.tile_pool(name="aaug", bufs=2))
        rawp = ctx.enter_context(tc.tile_pool(name="araw", bufs=2))
        sqbp = ctx.enter_context(tc.tile_pool(name="sqb", bufs=2))
        sqap = ctx.enter_context(tc.tile_pool(name="sqa", bufs=2))
        stage = ctx.enter_context(tc.tile_pool(name="stage", bufs=3))
        mpsum = ctx.enter_context(tc.tile_pool(name="mpsum", bufs=3, space="PSUM"))
        xpsum = ctx.enter_context(tc.tile_pool(name="xpsum", bufs=2, space="PSUM"))

        sel = const.tile([C + 1, 4], F32R)
        nc.sync.dma_start(out=sel[:, :], in_=sel_d[:, :])

        copy_tick = 0
        for bt in range(B):
            # ---- rhs: b_aug [C+2, M] = [b ; 1 ; bb] ----
            baug = bpool.tile([KAUG, M], F32R, tag="baug")
            nc.gpsimd.dma_start(out=baug[0:C, :], in_=b_d[bt])
            sqb = sqbp.tile([C + 1, M], F32R, tag="sqb")
            nc.scalar.square(sqb[0:C, :], baug[0:C, :].bitcast(F32))
            nc.gpsimd.dma_start(out=sqb[C : C + 1, :], in_=ones_d[:][None, :])
            for j in range(M // MC):
                pb = xpsum.tile([2, MC], F32, tag="xp")
                nc.tensor.matmul(
                    pb[:, :],
                    sel[:, 0:2],
                    sqb[:, j * MC : (j + 1) * MC],
                )
                dst = baug[C : C + 2, j * MC : (j + 1) * MC]
                if copy_tick % 2 == 0:
                    nc.vector.tensor_copy(dst, pb[:, :])
                else:
                    nc.scalar.copy(dst, pb[:, :])
                copy_tick += 1

            # ---- lhsT: a_aug [C+2, NSH] = [-2a ; aa ; 1] ----
            araw = rawp.tile([C, NSH], F32, tag="araw")
            nc.gpsimd.dma_start(out=araw[:, :], in_=a_d[bt])
            aaug = apool.tile([KAUG, NSH], F32R, tag="aaug")
            nc.vector.tensor_scalar_mul(aaug[0:C, :], araw[:, :], -2.0)
            sqa = sqap.tile([C + 1, NSH], F32R, tag="sqa")
            nc.scalar.square(sqa[0:C, :], araw[:, :])
            nc.gpsimd.dma_start(out=sqa[C : C + 1, :], in_=ones_d[0:NSH][None, :])
            pa = xpsum.tile([2, NSH], F32, tag="xp")
            nc.tensor.matmul(
                pa[:, :],
                sel[:, 2:4],
                sqa[:, :],
            )
            nc.vector.tensor_copy(aaug[C : C + 2, :], pa[:, :])

            # ---- main matmuls: d tile [128, 512] per PE pass ----
            for i in range(NB):
                st = stage.tile([128, M], F32, tag="st")
                for jj in range(M // PSUM_W):
                    pt = mpsum.tile([128, PSUM_W], F32, tag="mp")
                    for h in range(PSUM_W // MC):
                        col = jj * PSUM_W + h * MC
                        nc.tensor.matmul(
                            pt[:, h * MC : (h + 1) * MC],
                            aaug[:, i * 128 : (i + 1) * 128],
                            baug[:, col : col + MC],
                        )
                    dst = st[:, jj * PSUM_W : (jj + 1) * PSUM_W]
                    # slight DVE bias: ACT also does the squares
                    if copy_tick % 16 < 9:
                        nc.vector.tensor_copy(dst, pt[:, :])
                    else:
                        nc.scalar.copy(dst, pt[:, :])
                    copy_tick += 1
                nc.sync.dma_start(
                    out=d_d[bt, i * 128 : (i + 1) * 128, :], in_=st[:, :]
                )

    nc.compile()
    return nc


def _get_nc():
    if "nc" not in _CACHE:
        _CACHE["nc"] = _build_nc()
    return _CACHE["nc"]


def _make_sel():
    sel = np.zeros([C + 1, 4], dtype=np.float32)
    sel[C, 0] = 1.0      # out row 0 = sq[C] = ones row      (b side)
    sel[0:C, 1] = 1.0    # out row 1 = sum_c sq[c] = bb      (b side)
    sel[0:C, 2] = 1.0    # out row 0 = aa                    (a side)
    sel[C, 3] = 1.0      # out row 1 = ones                  (a side)
    return sel


def _make_in_maps(a, b):
    a = np.ascontiguousarray(np.asarray(a, dtype=np.float32))
    b = np.ascontiguousarray(np.asarray(b, dtype=np.float32))
    ones = np.ones([M], dtype=np.float32)
    sel = _make_sel()
    in_maps = []
    for k in range(NCORES):
        in_maps.append(
            {
                "a": np.ascontiguousarray(a[:, :, k * NSH : (k + 1) * NSH]),
                "b": b,
                "ones": ones,
                "sel": sel,
            }
        )
    return in_maps


def kernel(a, b, _trace=False, _trace_kwargs=None):
    nc = _get_nc()
    in_maps = _make_in_maps(a, b)
    res = run_bass_kernel_spmd(
        nc,
        in_maps,
        core_ids=list(range(NCORES)),
        trace=_trace,
        **(_trace_kwargs or {}),
    )
    out = np.concatenate([res.results[k]["d"] for k in range(NCORES)], axis=1)
    if _trace:
        _CACHE["last_results"] = res
    return out


# revision 4
# speedup vs baseline: 1.0247x; 1.0247x over previous
"""Pairwise squared-Euclidean distance map on 8 TRN2 NeuronCores.

d[b, i, j] = sum_c (a[b, c, i] - b[b, c, j])^2
           = aa[b, i] + bb[b, j] - 2 * <a[b, :, i], b[b, :, j]>

Sharding: data-parallel over the N dimension (rows of the distance map).
Core k computes d[:, k*512:(k+1)*512, :] from a[:, :, k*512:(k+1)*512]
and the full (small) b tensor.

Per-core kernel: one augmented bf16 matmul per output tile computes the
full expression in a single PE pass (bf16 streams 1 column/cycle vs ~3
for fp32r on TRN2). With centered norm rows (E[|a_i|^2] = C = 64, so
aa-64 / bb-64 are ~4x smaller in magnitude, shrinking their bf16
rounding error) the augmentation is
    lhsT = [ -2*a ; aa-64 ;  1  ; 128 ]   (K = C+3 = 67, 128 N cols)
    rhs  = [   b  ;   1   ; bb-64 ; 1 ]   (67 partitions, 512 M cols)
so lhsT.T @ rhs = -2*a.b + (aa-64) + (bb-64) + 128 = d, accumulated in
fp32 PSUM. Post-processing is a plain PSUM->SBUF fp32 copy (split
across the Vector and Scalar engines) before the store DMA.

The three augmented rows per side are produced by ONE selector matmul:
square the inputs (ScalarE, bf16 out), append a ones row at partition
64, contract with a constant [65, 3] selector (columns pick the ones
row, sum the squares minus 64, or scale the ones row by 128) -> PSUM
[3, 512] -> a single base-64 copy into the augmented tile. (Engine APs
may only start at partitions {0, 32, 64, 96}.)
"""

import numpy as np
from contextlib import ExitStack

import concourse.bass as bass
import concourse.bacc as bacc
import concourse.mybir as mybir
from concourse.tile import TileContext
from concourse.bass_utils import run_bass_kernel_spmd

B, C, N, M = 4, 64, 4096, 4096
NCORES = 8
NSH = N // NCORES          # 512 N rows per core
NB = NSH // 128            # 4 row blocks of 128
MC = 512                   # matmul moving free dim (one PSUM bank of fp32)
PSUM_W = 1024              # main PSUM tile width (2 banks, 2 matmuls)
KAUG = C + 3               # contraction dim with the three norm rows

F32 = mybir.dt.float32
BF16 = mybir.dt.bfloat16

_CACHE = {}


def _build_nc():
    nc = bacc.Bacc(
        "TRN2",
        target_bir_lowering=False,
        debug=False,
        enable_asserts=True,
        num_devices=NCORES,
    )
    a_d = nc.declare_dram_parameter("a", [B, C, NSH], F32, isOutput=False)
    b_d = nc.declare_dram_parameter("b", [B, C, M], F32, isOutput=False)
    ones_d = nc.declare_dram_parameter("ones", [M], BF16, isOutput=False)
    # selector columns (see _make_sel): per side [pick-ones | sum-sq - 64 |
    # 128 * pick-ones] arranged for the row order each side needs
    sel_d = nc.declare_dram_parameter("sel", [C + 1, 6], BF16, isOutput=False)
    d_d = nc.declare_dram_parameter("d", [B, NSH, M], F32, isOutput=True)

    with ExitStack() as ctx:
        tc = ctx.enter_context(TileContext(nc))
        const = ctx.enter_context(tc.tile_pool(name="const", bufs=1))
        bpool = ctx.enter_context(tc.tile_pool(name="baug", bufs=2))
        apool = ctx.enter_context(tc.tile_pool(name="aaug", bufs=2))
        rawp = ctx.enter_context(tc.tile_pool(name="araw", bufs=2))
        sqbp = ctx.enter_context(tc.tile_pool(name="sqb", bufs=2))
        sqap = ctx.enter_context(tc.tile_pool(name="sqa", bufs=2))
        stage = ctx.enter_context(tc.tile_pool(name="stage", bufs=4))
        mpsum = ctx.enter_context(tc.tile_pool(name="mpsum", bufs=3, space="PSUM"))
        xpsum = ctx.enter_context(tc.tile_pool(name="xpsum", bufs=2, space="PSUM"))

        sel = const.tile([C + 1, 6], BF16)
        nc.sync.dma_start(out=sel[:, :], in_=sel_d[:, :])

        copy_tick = 0
        for bt in range(B):
            # ---- rhs: b_aug [C+3, M] = [b ; 1 ; bb-64 ; 1] (bf16) ----
            baug = bpool.tile([KAUG, M], BF16, tag="baug")
            nc.gpsimd.dma_start(out=baug[0:C, :], in_=b_d[bt])  # f32->bf16 cast
            sqb = sqbp.tile([C + 1, M], BF16, tag="sqb")
            nc.scalar.square(sqb[0:C, :], baug[0:C, :])
            nc.gpsimd.dma_start(out=sqb[C : C + 1, :], in_=ones_d[:][None, :])
            for j in range(M // MC):
                pb = xpsum.tile([3, MC], F32, tag="xp")
                nc.tensor.matmul(
                    pb[:, :],
                    sel[:, 0:3],
                    sqb[:, j * MC : (j + 1) * MC],
                )
                dst = baug[C : C + 3, j * MC : (j + 1) * MC]
                if copy_tick % 2 == 0:
                    nc.vector.tensor_copy(dst, pb[:, :])
                else:
                    nc.scalar.copy(dst, pb[:, :])
                copy_tick += 1

            # ---- lhsT: a_aug [C+3, NSH] = [-2a ; aa-64 ; 1 ; 128] ----
            araw = rawp.tile([C, NSH], F32, tag="araw")
            nc.gpsimd.dma_start(out=araw[:, :], in_=a_d[bt])
            aaug = apool.tile([KAUG, NSH], BF16, tag="aaug")
            nc.vector.tensor_scalar_mul(aaug[0:C, :], araw[:, :], -2.0)
            sqa = sqap.tile([C + 1, NSH], BF16, tag="sqa")
            nc.scalar.square(sqa[0:C, :], araw[:, :])
            nc.gpsimd.dma_start(out=sqa[C : C + 1, :], in_=ones_d[0:NSH][None, :])
            pa = xpsum.tile([3, NSH], F32, tag="xp")
            nc.tensor.matmul(
                pa[:, :],
                sel[:, 3:6],
                sqa[:, :],
            )
            nc.vector.tensor_copy(aaug[C : C + 3, :], pa[:, :])

            # ---- main matmuls: d tile [128, 512] per PE pass ----
            for i in range(NB):
                st = stage.tile([128, M], F32, tag="st")
                for jj in range(M // PSUM_W):
                    pt = mpsum.tile([128, PSUM_W], F32, tag="mp")
                    for h in range(PSUM_W // MC):
                        col = jj * PSUM_W + h * MC
                        nc.tensor.matmul(
                            pt[:, h * MC : (h + 1) * MC],
                            aaug[:, i * 128 : (i + 1) * 128],
                            baug[:, col : col + MC],
                        )
                    dst = st[:, jj * PSUM_W : (jj + 1) * PSUM_W]
                    # slight DVE bias: ACT also does the squares
                    if copy_tick % 16 < 9:
                        nc.vector.tensor_copy(dst, pt[:, :])
                    else:
                        nc.scalar.copy(dst, pt[:, :])
                    copy_tick += 1
                nc.sync.dma_start(
                    out=d_d[bt, i * 128 : (i + 1) * 128, :], in_=st[:, :]
                )

    nc.compile()
    return nc


def _get_nc():
    if "nc" not in _CACHE:
        _CACHE["nc"] = _build_nc()
    return _CACHE["nc"]


def _make_sel():
    sel = np.zeros([C + 1, 6], dtype=np.float32)
    # b side -> baug rows [C..C+2] = [ones ; bb-64 ; ones]
    sel[C, 0] = 1.0
    sel[0:C, 1] = 1.0
    sel[C, 1] = -64.0
    sel[C, 2] = 1.0
    # a side -> aaug rows [C..C+2] = [aa-64 ; ones ; 128]
    sel[0:C, 3] = 1.0
    sel[C, 3] = -64.0
    sel[C, 4] = 1.0
    sel[C, 5] = 128.0
    return sel


def _bf16(x):
    import ml_dtypes

    return np.asarray(x).astype(ml_dtypes.bfloat16)


def _make_in_maps(a, b):
    a = np.ascontiguousarray(np.asarray(a, dtype=np.float32))
    b = np.ascontiguousarray(np.asarray(b, dtype=np.float32))
    ones = _bf16(np.ones([M], dtype=np.float32))
    sel = _bf16(_make_sel())
    in_maps = []
    for k in range(NCORES):
        in_maps.append(
            {
                "a": np.ascontiguousarray(a[:, :, k * NSH : (k + 1) * NSH]),
                "b": b,
                "ones": ones,
                "sel": sel,
            }
        )
    return in_maps


def kernel(a, b, _trace=False, _trace_kwargs=None):
    nc = _get_nc()
    in_maps = _make_in_maps(a, b)
    res = run_bass_kernel_spmd(
        nc,
        in_maps,
        core_ids=list(range(NCORES)),
        trace=_trace,
        **(_trace_kwargs or {}),
    )
    out = np.concatenate([res.results[k]["d"] for k in range(NCORES)], axis=1)
    if _trace:
        _CACHE["last_results"] = res
    return out
